# revision 15
# baseline (speedup 1.0000x reference)
"""Multi-head causal attention (B=4, S=2048, H=16, D=64) on 8 trn2 cores, v2.

Sharding: core c -> (batch b = c//2, head-half hh = c%2, heads 8hh..8hh+7).
Each core: Q/K/V projections for its 8 heads (column-sharded weights), full
causal attention over all 2048 query rows for those heads, and a partial
output projection (row-sharded Wo). Host sums the two partials per batch and
adds bo (the "all-reduce" of the sharding hint, done in the unshard step).

Host-side prep: q/k/v are transposed and cast to bf16 on the host, so the
device needs no input transposes at all; weights are pre-sliced, pre-swizzled
d-tile-major so each is a single 1MB DMA.

Attention produces its output directly in transposed layout (j on
partitions): the AV matmul uses projected-V (plus a ones column for the
softmax denominator) as the stationary operand and the exp'd scores as the
256-wide moving operand. Normalization multiplies by a PE-broadcast
reciprocal row. Causal masking is a multiplicative bf16 {0,1} mask applied
to the exp'd scores only on the two diagonal-straddling k-blocks of each
256-row query chunk.
"""

import numpy as np
import ml_dtypes

import concourse.bass as bass
import concourse.mybir as mybir
from concourse.tile import TileContext
from concourse.bass_utils import run_bass_kernel_spmd

F32 = mybir.dt.float32
BF16 = mybir.dt.bfloat16

B, S, H, D = 4, 2048, 16, 64
DM = H * D            # 1024
HPC = 8               # heads per core
JH = HPC * D          # 512 = per-core projection width
QCH = 256             # query chunk rows
NT = S // QCH         # 8 chunks
KB = 128              # k block rows
N_CORES = 8

_bf = ml_dtypes.bfloat16


def _split_excess_waits(nc):
    """walrus on this stack accepts at most ONE semaphore wait per
    instruction; Tile emits more on drains/branches/etc. Move excess
    waits onto preceding same-engine nops (semantically identical: the
    engine blocks on the nops first)."""
    for f in nc.m.functions:
        for bb in f.blocks:
            new_instrs = []
            for ins in bb.instructions:
                si = ins.sync_info
                if si is not None and si.on_wait is not None and len(si.on_wait) > 1:
                    waits = list(si.on_wait)
                    extra, keep = waits[:-1], waits[-1:]
                    for i, w in enumerate(extra):
                        new_instrs.append(mybir.InstNoOp(
                            name=f"{ins.name}-ws{i}", engine=ins.engine,
                            ins=[], outs=[],
                            sync_info=mybir.SyncInfo(on_wait=[w], on_update=[])))
                    ins.sync_info = mybir.SyncInfo(on_wait=keep,
                                                   on_update=list(si.on_update))
                new_instrs.append(ins)
            bb.instructions[:] = new_instrs


def build_mha(masking: bool, repeat: int = 1, do_proj: bool = True,
              do_attn: bool = True, do_o: bool = True):
    nc = bass.Bass()

    qT_in = nc.dram_tensor("qT", [DM, S], BF16, kind="ExternalInput")
    kT_in = nc.dram_tensor("kT", [DM, S], BF16, kind="ExternalInput")
    vT_in = nc.dram_tensor("vT", [DM, S], BF16, kind="ExternalInput")
    wq_d = nc.dram_tensor("wq", [128, 8 * JH], BF16, kind="ExternalInput")
    wk_d = nc.dram_tensor("wk", [128, 8 * JH], BF16, kind="ExternalInput")
    wv_d = nc.dram_tensor("wv", [128, 8 * JH], BF16, kind="ExternalInput")
    wo_d = nc.dram_tensor("wo", [128, 4 * DM], BF16, kind="ExternalInput")
    bq_d = nc.dram_tensor("bq", [128, 4], F32, kind="ExternalInput")
    bk_d = nc.dram_tensor("bk", [128, 4], F32, kind="ExternalInput")
    bv_d = nc.dram_tensor("bv", [1, JH], BF16, kind="ExternalInput")
    msk_d = nc.dram_tensor("msk", [128, 2 * QCH], BF16, kind="ExternalInput")
    out = nc.dram_tensor("out", [S, DM], BF16, kind="ExternalOutput")

    for _rep in range(repeat):
      with TileContext(nc) as tc:
        with (tc.tile_pool(name="persist", bufs=1) as pp,
              tc.tile_pool(name="stg", bufs=1) as stg,
              tc.tile_pool(name="work", bufs=1) as wk,
              tc.tile_pool(name="ps", bufs=1, space="PSUM") as ps):
            # ---- constants ----
            ones_sb = pp.tile([1, 128], BF16, tag="ones")
            nc.vector.memset(ones_sb[:], 1.0)
            bq_sb = pp.tile([128, 4], F32, tag="bq")
            bk_sb = pp.tile([128, 4], F32, tag="bk")
            bv_sb = pp.tile([1, JH], BF16, tag="bv")
            msk_sb = pp.tile([128, 2 * QCH], BF16, tag="msk")
            nc.gpsimd.dma_start(bq_sb[:], bq_d[:])
            nc.gpsimd.dma_start(bk_sb[:], bk_d[:])
            nc.gpsimd.dma_start(bv_sb[:], bv_d[:])
            nc.gpsimd.dma_start(msk_sb[:], msk_d[:])

            # ---- weights (one 1MB DMA each; d-tile-major swizzled) ----
            wq_sb = pp.tile([128, 8 * JH], BF16, tag="wq", name="wq_sb")
            wk_sb = pp.tile([128, 8 * JH], BF16, tag="wk", name="wk_sb")
            wv_sb = pp.tile([128, 8 * JH], BF16, tag="wv", name="wv_sb")
            wo_sb = pp.tile([128, 4 * DM], BF16, tag="wo", name="wo_sb")
            nc.scalar.dma_start(wk_sb[:], wk_d[:])
            nc.scalar.dma_start(wq_sb[:], wq_d[:])
            nc.scalar.dma_start(wv_sb[:], wv_d[:])

            # ---- persistent activations ----
            kpT = [pp.tile([128, S], BF16, tag=f"kpT{i}", name=f"kpT{i}")
                   for i in range(4)]
            qpT = [pp.tile([128, S], BF16, tag=f"qpT{i}", name=f"qpT{i}")
                   for i in range(4)]
            v_sb = [pp.tile([128, HPC * (D + 1)], BF16, tag=f"v{t}",
                            name=f"v{t}") for t in range(S // KB)]
            attnT = [pp.tile([128, S], BF16, tag=f"attnT{i}", name=f"attnT{i}")
                     for i in range(4)]

            # ---- input staging: column-half slices (2 per d-tile) ----
            # DMA issue order tracks consumption order: sl=0 K then V on the
            # sync queue; wk/wq/wv then sl=0 Q then wo on the scalar queue;
            # all sl=1 slices last on both queues.
            kin, qin, vin = {}, {}, {}
            if do_proj:
                for sl in range(2):
                    cs = slice(sl * 1024, (sl + 1) * 1024)
                    for j in range(8):
                        kin[sl, j] = stg.tile([128, 1024], BF16, tag="in",
                                              bufs=32, name=f"kin{sl}_{j}")
                        nc.sync.dma_start(kin[sl, j][:],
                                          kT_in[j * 128:(j + 1) * 128, cs])
                    for j in range(8):
                        vin[sl, j] = stg.tile([128, 1024], BF16, tag="in",
                                              bufs=32, name=f"vin{sl}_{j}")
                        nc.sync.dma_start(vin[sl, j][:],
                                          vT_in[j * 128:(j + 1) * 128, cs])
                    for j in range(8):
                        qin[sl, j] = stg.tile([128, 1024], BF16, tag="in",
                                              bufs=32, name=f"qin{sl}_{j}")
                        nc.scalar.dma_start(qin[sl, j][:],
                                            qT_in[j * 128:(j + 1) * 128, cs])
                    if sl == 0:
                        nc.scalar.dma_start(wo_sb[:], wo_d[:])
            else:
                nc.scalar.dma_start(wo_sb[:], wo_d[:])

            def kq_chain(rc, i, win, w_sb, b_sb, outT):
                sl, c0 = rc // 2, (rc % 2) * 512
                p = ps.tile([128, 512], F32, tag="pp", bufs=2)
                for jd in range(8):
                    nc.tensor.matmul(
                        p[:],
                        w_sb[:, jd * JH + i * 128:jd * JH + (i + 1) * 128],
                        win[sl, jd][:, c0:c0 + 512],
                        start=(jd == 0), stop=(jd == 7))
                nc.vector.tensor_scalar_add(
                    outT[i][:, rc * 512:(rc + 1) * 512], p[:],
                    b_sb[:, i:i + 1])

            def v_proj(st):
                # V: row-major [k, j] + ones column per head
                sl, c0 = st // 8, (st % 8) * 128
                v3 = v_sb[st].rearrange("p (h x) -> p h x", x=D + 1)
                nc.vector.memset(v3[:, :, 64:65], 1.0)
                p = ps.tile([128, 512], F32, tag="pp", bufs=2)
                for jd in range(8):
                    nc.tensor.matmul(
                        p[:], vin[sl, jd][:, c0:c0 + 128],
                        wv_sb[:, jd * JH:(jd + 1) * JH],
                        start=(jd == 0), stop=False)
                nc.tensor.matmul(p[:], ones_sb[:], bv_sb[:],
                                 start=False, stop=True)
                nc.vector.tensor_copy(v3[:, :, 0:64], p[:])

            def o_chain(qb, r2):
                ot = wk.tile([128, DM], BF16, tag="ot", bufs=4)
                for mc in range(2):
                    po = ps.tile([128, 512], F32, tag="pp", bufs=2)
                    for jt in range(4):
                        nc.tensor.matmul(
                            po[:],
                            attnT[jt][:, qb * 128:(qb + 1) * 128],
                            wo_sb[:, jt * DM + mc * 512:
                                  jt * DM + (mc + 1) * 512],
                            start=(jt == 0), stop=(jt == 3))
                    with nc.allow_low_precision(
                            reason="bf16 partial O output, summed on host"):
                        nc.vector.tensor_copy(
                            ot[:, mc * 512:(mc + 1) * 512], po[:])
                eng = nc.sync if r2 == 0 else nc.scalar
                eng.dma_start(out[qb * 128:(qb + 1) * 128, :], ot[:])

            # ---- filler queue: projection/O chains with emission deadlines.
            # K(rc,i) must be emitted before chunk 2rc reads kpT[i]; Q(rc,i)
            # before chunk 2rc; V(st) before chunk st//2. Deadline fractions
            # order chains within a chunk. O chains have no deadline (99).
            fillers = []
            if do_proj:
                for rc in range(4):
                    for i in range(4):
                        fillers.append(
                            (2 * rc + 0.01 * i,
                             lambda rc=rc, i=i: kq_chain(rc, i, kin, wk_sb,
                                                         bk_sb, kpT)))
                        fillers.append(
                            (2 * rc + 0.1 + 0.01 * i,
                             lambda rc=rc, i=i: kq_chain(rc, i, qin, wq_sb,
                                                         bq_sb, qpT)))
                for st in range(S // KB):
                    fillers.append((st // 2 + 0.2 + 0.001 * st,
                                    lambda st=st: v_proj(st)))
            fillers.sort(key=lambda f: f[0])

            def pop_fillers(n):
                for _ in range(n):
                    if not fillers:
                        return
                    fillers.pop(0)[1]()

            def pop_due(t):
                n = 0
                while fillers and fillers[0][0] < t + 1:
                    fillers.pop(0)[1]()
                    n += 1
                return n

            # non-wavefront path (no causal attention interleave)
            if not (do_attn and masking):
                pop_fillers(len(fillers))

            # Per-chunk opportunistic filler quota (chains). Attention alone
            # is ACT-bound at every chunk (exp > QK+AV on PE), so fillers
            # flow inside the attention stretches; cumulative quota stays
            # ahead of the next chunk's projection deadlines so the forced
            # pops at chunk starts (which leave ACT idle) stay empty.
            QUOTA = [4, 8, 8, 10, 9, 9, 8, 8]

            # ---- attention (causal; q chunks of 256) ----
            if do_attn:
                for t in range(NT):
                    pop_due(t) if masking else 0
                    quota = QUOTA[t] if masking else 0
                    G0 = (2 * (t + 1) + 3) // 4 if masking else 4
                    tot_pts = 8 * G0 + 4
                    pace = [0, 0]  # points used, opp pops done

                    def opp_pop():
                        # pace quota evenly across this chunk's pop points
                        pace[0] += 1
                        tgt = quota * pace[0] // tot_pts
                        n = min(tgt - pace[1], len(fillers))
                        if n > 0:
                            pop_fillers(n)
                            pace[1] += n

                    nkb = 2 * (t + 1) if masking else S // KB
                    G = (nkb + 3) // 4
                    atus = []
                    # phase 1: scores -> exp -> AV accumulate -> fast evict.
                    # Heads are processed in pairs occupying complementary
                    # 64-row groups of the PE array so their QK matmuls
                    # overlap (row tiling).
                    for hp in range(HPC // 2):
                        ats, exs = [], []
                        for h in (2 * hp, 2 * hp + 1):
                            ats.append(ps.tile([128, 2 * QCH], F32, tag="at",
                                               bufs=2, name="at"))
                        for g in range(G):
                            kbs = list(range(4 * g, min(4 * g + 4, nkb)))
                            n = len(kbs)
                            exs = []
                            for hi, h in enumerate((2 * hp, 2 * hp + 1)):
                                ho = (h % 2) * 64
                                sc = ps.tile([128, 1024], F32, tag="sc",
                                             bufs=2)
                                for jj, kb in enumerate(kbs):
                                    nc.tensor.matmul(
                                        sc[:, jj * QCH:(jj + 1) * QCH],
                                        kpT[hp][ho:ho + 64,
                                                kb * 128:(kb + 1) * 128],
                                        qpT[hp][ho:ho + 64,
                                                t * QCH:(t + 1) * QCH],
                                        start=True, stop=True)
                                ex = wk.tile([128, 1024], BF16, tag="ex",
                                             bufs=6)
                                nc.scalar.activation(
                                    ex[:, 0:n * QCH], sc[:, 0:n * QCH],
                                    mybir.ActivationFunctionType.Exp,
                                    scale=0.125)
                                if masking and g == G - 1:
                                    nc.vector.tensor_mul(
                                        ex[:, (n - 2) * QCH:n * QCH],
                                        ex[:, (n - 2) * QCH:n * QCH],
                                        msk_sb[:])
                                exs.append(ex)
                            opp_pop()
                            for hi, h in enumerate((2 * hp, 2 * hp + 1)):
                                for jj, kb in enumerate(kbs):
                                    nc.tensor.matmul(
                                        ats[hi][0:65, 0:QCH],
                                        v_sb[kb][:, 65 * h:65 * h + 65],
                                        exs[hi][:, jj * QCH:(jj + 1) * QCH],
                                        start=(g == 0 and jj == 0),
                                        stop=(g == G - 1 and jj == n - 1))
                            opp_pop()
                        for hi in range(2):
                            atu = wk.tile([65, QCH], BF16, tag="atu",
                                          bufs=10, name="atu")
                            src = ats[hi][0:65, 0:QCH]
                            if hi == 0:
                                nc.scalar.copy(atu[:], src)
                            else:
                                with nc.allow_low_precision(
                                        reason="bf16 softmax num/denom"):
                                    nc.vector.tensor_copy(atu[:], src)
                            atus.append(atu)
                    # phase 2: batched normalization (off the PSUM/PE
                    # critical path; reciprocals are all ready by now)
                    for hp in range(HPC // 2):
                        rec = wk.tile([1, 2 * QCH], BF16, tag="rec", bufs=8)
                        for hi in range(2):
                            with nc.allow_low_precision(
                                    reason="bf16 softmax recip, ~0.4% rel"):
                                nc.vector.reciprocal(
                                    rec[:, hi * QCH:(hi + 1) * QCH],
                                    atus[2 * hp + hi][64:65, :])
                        rb = ps.tile([128, 2 * QCH], F32, tag="at", bufs=2,
                                     name="rb")
                        nc.tensor.matmul(rb[0:64, :], ones_sb[:, 0:64],
                                         rec[:], start=True, stop=True)
                        for hi in range(2):
                            h = 2 * hp + hi
                            ht, ho = h // 2, (h % 2) * 64
                            with nc.allow_low_precision(
                                    reason="bf16 softmax normalize"):
                                nc.vector.tensor_mul(
                                    attnT[ht][ho:ho + 64,
                                              t * QCH:(t + 1) * QCH],
                                    atus[h][0:64, :],
                                    rb[0:64, hi * QCH:(hi + 1) * QCH])
                        opp_pop()

                    # ---- output projection: deferred into filler queue
                    if do_o:
                        for r2 in range(2):
                            fillers.append(
                                (99, lambda qb=2 * t + r2, r2=r2:
                                 o_chain(qb, r2)))
                pop_fillers(len(fillers))

    _split_excess_waits(nc)
    return nc


def _host_prep(query, key, value, Wq, bq, Wk, bk, Wv, bv, Wo, bo):
    """Host-side shard prep. Returns (in_maps list of 8, bo f32)."""
    def t_in(x):   # [S, DM] f32 -> [DM, S] bf16 contiguous
        return np.ascontiguousarray(np.asarray(x, np.float32).T).astype(_bf)

    def w_sw(WT_half, blocks, width):  # [128*blocks, width] -> [128, blocks*width]
        return np.ascontiguousarray(
            WT_half.reshape(blocks, 128, width).transpose(1, 0, 2)
            .reshape(128, blocks * width))

    WqT = np.asarray(Wq, np.float32).T.astype(_bf)
    WkT = np.asarray(Wk, np.float32).T.astype(_bf)
    WvT = np.asarray(Wv, np.float32).T.astype(_bf)
    WoT = np.asarray(Wo, np.float32).T.astype(_bf)
    bq = np.asarray(bq, np.float32)
    bk = np.asarray(bk, np.float32)
    bv = np.asarray(bv, np.float32)

    # mask for the two diagonal-straddling k-blocks of a 256-row q chunk
    dk = np.arange(128)[:, None]
    dq = np.arange(QCH)[None, :]
    msk = np.concatenate([(dk <= dq), (128 + dk <= dq)], axis=1).astype(_bf)

    halves = []
    for hh in range(2):
        js = slice(hh * JH, (hh + 1) * JH)
        halves.append({
            "wq": w_sw(WqT[:, js], 8, JH),
            "wk": w_sw(WkT[:, js], 8, JH),
            "wv": w_sw(WvT[:, js], 8, JH),
            "wo": w_sw(WoT[js, :], 4, DM),
            "bq": np.ascontiguousarray(bq[js].reshape(4, 128).T),
            "bk": np.ascontiguousarray(bk[js].reshape(4, 128).T),
            "bv": bv[js].reshape(1, JH).astype(_bf),
            "msk": msk,
        })

    in_maps = []
    for c in range(N_CORES):
        b, hh = c // 2, c % 2
        m = dict(halves[hh])
        m["qT"] = t_in(query[b])
        m["kT"] = t_in(key[b])
        m["vT"] = t_in(value[b])
        in_maps.append(m)
    return in_maps, np.asarray(bo, np.float32)


def gather_out(core_outs, bo):
    out = np.empty((B, S, DM), np.float32)
    for b in range(B):
        out[b] = (np.asarray(core_outs[2 * b], dtype=np.float32) +
                  np.asarray(core_outs[2 * b + 1], dtype=np.float32) + bo)
    return out


_CACHE = {}


def kernel(query, key, value, Wq, bq, Wk, bk, Wv, bv, Wo, bo, masking):
    masking = bool(int(np.asarray(masking)))
    if masking not in _CACHE:
        _CACHE[masking] = build_mha(masking)
    nc = _CACHE[masking]
    in_maps, bo_f = _host_prep(query, key, value, Wq, bq, Wk, bk,
                               Wv, bv, Wo, bo)
    res = run_bass_kernel_spmd(nc, in_maps, list(range(N_CORES)))
    return gather_out([r["out"] for r in res.results], bo_f)

